# revision 2
# baseline (speedup 1.0000x reference)
import sys, time
sys.path.insert(0, "/opt/trn_rl_repo")
import numpy as np
import ml_dtypes
from contextlib import ExitStack

import concourse.bass as bass
import concourse.tile as tile
import concourse.bass_isa as bass_isa
from concourse import mybir, bacc
from concourse.bass_utils import run_bass_kernel_spmd

BF16 = ml_dtypes.bfloat16
F8 = ml_dtypes.float8_e4m3
F32 = mybir.dt.float32
BF = mybir.dt.bfloat16
E4 = mybir.dt.float8e4
AF = mybir.ActivationFunctionType
OP = mybir.AluOpType
RED = bass_isa.ReduceOp

B, L, DM, ED, EDH, N, DT_RANK, NL = 4, 1024, 512, 1024, 512, 16, 32, 2
EPS = 1e-5

REPEAT = 1
LAST_RUN_S = 0.0
ABLATE = frozenset()
_CACHE = {}


def _build(repeat, kvals, has_cvb, mode=frozenset()):
    nc = bacc.Bacc("TRN2", target_bir_lowering=False, debug=False, num_devices=8)
    xT_d = nc.dram_tensor("xT", [128, 4096], BF, kind="ExternalInput")
    winT_d = nc.dram_tensor("winT", [128, 16384], E4, kind="ExternalInput")
    wout_d = nc.dram_tensor("wout", [128, 8192], E4, kind="ExternalInput")
    wxp_d = nc.dram_tensor("wxp", [128, 1024], E4, kind="ExternalInput")
    wdt_d = nc.dram_tensor("wdt", [33, 2048], BF, kind="ExternalInput")
    cvw_d = nc.dram_tensor("cvw", [128, 64], F32, kind="ExternalInput")
    cvb_d = nc.dram_tensor("cvb", [128, 16], F32, kind="ExternalInput")
    Dv_d = nc.dram_tensor("Dv", [128, 16], F32, kind="ExternalInput")
    kvi_d = nc.dram_tensor("kvi", [128, 32], F32, kind="ExternalInput")
    fcp_d = nc.dram_tensor("fcp", [128, 4], F32, kind="ExternalInput")
    fcb_d = nc.dram_tensor("fcb", [1, 1], F32, kind="ExternalInput")
    out_d = nc.dram_tensor("out", [1, 1024], F32, kind="ExternalOutput")
    bcb_d = nc.dram_tensor("bcb", [32, 1024], BF)   # local B/C bounce

    with tile.TileContext(nc) as tc, ExitStack() as ctx:
        sb = ctx.enter_context(tc.tile_pool(name="sb", bufs=1))
        pp = ctx.enter_context(
            tc.tile_pool(name="pp", bufs=1, space=bass.MemorySpace.PSUM))

        # ---- persistent weights ----
        winT = sb.tile([128, 16384], E4)
        nc.sync.dma_start(winT[:], winT_d[:])
        wout = sb.tile([128, 8192], E4)
        nc.sync.dma_start(wout[:], wout_d[:])
        wxp = sb.tile([128, 1024], E4)
        nc.sync.dma_start(wxp[:], wxp_d[:])
        wdt = sb.tile([33, 2048], BF)
        nc.sync.dma_start(wdt[:], wdt_d[:])
        cvw = sb.tile([128, 64], F32)
        nc.sync.dma_start(cvw[:], cvw_d[:])
        cvb = sb.tile([128, 16], F32)
        nc.sync.dma_start(cvb[:], cvb_d[:])
        Dv = sb.tile([128, 16], F32)
        nc.sync.dma_start(Dv[:], Dv_d[:])
        kvi = sb.tile([128, 32], F32)
        nc.sync.dma_start(kvi[:], kvi_d[:])
        fcp = sb.tile([128, 4], F32)
        nc.sync.dma_start(fcp[:], fcp_d[:])
        fcb = sb.tile([1, 1], F32)
        nc.sync.dma_start(fcb[:], fcb_d[:])
        epsc = sb.tile([128, 1], F32)
        nc.vector.memset(epsc[:], EPS)

        # ---- dynamic tiles ----
        xP = sb.tile([128, 4096], BF)
        nc.sync.dma_start(xP[:], xT_d[:])
        xW = sb.tile([128, 4096], BF)        # residual stream
        xq = sb.tile([128, 4096], E4)        # rms-x fp8 (in_proj moving)
        xinpad = sb.tile([128, 8216], E4)    # conv input x8: 8 x [3 | 1024]
        xinq = sb.tile([128, 8192], E4)      # silu(conv) true (full ED)
        sz = sb.tile([128, 8192], E4)        # silu(z)
        lu = sb.tile([128, 4096], BF)        # delta (one ED-half at a time)
        y3q = sb.tile([128, 8192], E4)       # gate out x256
        utc = sb.tile([128, 1024], BF)       # per-chunk ut scratch
        dbcdt = sb.tile([33, 1024], BF)      # dt moving (+ones row)
        bcst = sb.tile([64, 1024], BF)       # B/C evac staging (rows 32:64)
        BCmat = sb.tile([128, 32768], E4)    # B x8 | C x32
        dA = sb.tile([128, 16384], BF)
        dBxh = sb.tile([128, 16384], BF)
        dBf = dBxh[:].bitcast(F32)           # [128, 8192] f32 scratch view
        bob = sz[:].bitcast(BF)              # out_proj x64 (reuses sz bytes)

        nc.vector.memset(dbcdt[32:33, :], 1.0)
        nc.vector.memset(
            dA[:].rearrange("p (n l) -> p n l", n=N)[:, :, 0:1], 0.0)
        nc.vector.memset(
            xinpad[:].rearrange("p (m c) -> p m c", m=8)[:, :, 0:3], 0.0)

        MM = nc.tensor.matmul
        ACT = nc.scalar.activation
        TT = nc.vector.tensor_tensor
        TSM = nc.vector.tensor_scalar_mul
        PMDR = mybir.MatmulPerfMode.DoubleRow

        def nlv(t):
            return t[:].rearrange("p (n l) -> p n l", n=N)

        def mtv(ap, m=4):
            return ap.rearrange("p (m t) -> p m t", m=m)

        def bc_t(ap2d, n=1024):
            return ap2d.unsqueeze(2).broadcast_to([128, ap2d.shape[1], n])

        def xiv(kt, f):
            ap = xq[:, 2 * kt * 1024: 2 * (kt + 1) * 1024].rearrange(
                "p (i t) -> p i t", i=2)
            return ap[:, :, f * 512:(f + 1) * 512]

        def xinv(kt, f):
            # x_proj/out-style moving from xinq full-ED [128, 8192]
            ap = xinq[:, 2 * kt * 1024: 2 * (kt + 1) * 1024].rearrange(
                "p (i t) -> p i t", i=2)
            return ap[:, :, f * 512:(f + 1) * 512]

        def y3vv(kt, f):
            ap = y3q[:, 2 * kt * 1024: 2 * (kt + 1) * 1024].rearrange(
                "p (i t) -> p i t", i=2)
            return ap[:, :, f * 512:(f + 1) * 512]

        rep_ctx = tc.For_i(0, repeat)
        rep_ctx.__enter__()
        for _r in range(1):
            for li in range(NL):
                xT = xP if li == 0 else xW
                # ================= rmsnorm =================
                TT(dBf[:, 0:4096], xT[:], xT[:], OP.mult)
                nc.vector.tensor_reduce(
                    dBf[:, 4096:5120],
                    dBf[:, 0:4096].rearrange("p (dc t) -> p t dc", dc=4),
                    mybir.AxisListType.X, OP.add)
                nc.gpsimd.partition_all_reduce(
                    dBf[:, 5120:6144], dBf[:, 4096:5120], 128, RED.add)
                ACT(dBf[:, 4096:5120], dBf[:, 5120:6144],
                    AF.Abs_reciprocal_sqrt, scale=1.0 / DM, bias=epsc[:])
                TT(mtv(xq[:]), mtv(xT[:]),
                   dBf[:, 4096:5120].unsqueeze(1).broadcast_to([128, 4, 1024]),
                   OP.mult)

                xpv = xinpad[:].rearrange("p (m c) -> p m c", m=8)
                psx = pp.tile([128, 1024], F32, tag="px")
                for e in range(2):
                    # ============ in_proj xin-half (this e) ============
                    for m in range(4):
                        g = e * 4 + m      # global ED block 0..7
                        ps = pp.tile([128, 1024], F32, tag=f"p{m % 3}")
                        if "noin" in mode:
                            nc.vector.memset(ps[:], 0.0)
                        else:
                            for f in range(2):
                                for kt in range(2):
                                    off = ((li * 16 + g) * 2 + kt) * 256
                                    MM(ps[:, f * 512:(f + 1) * 512],
                                       winT[:, off:off + 256].rearrange(
                                           "p (i m) -> p i m", i=2),
                                       xiv(kt, f),
                                       start=(kt == 0), stop=(kt == 1),
                                       perf_mode=PMDR)
                        TSM(xpv[:, g:g + 1, 3:1027], ps[:].unsqueeze(1),
                            8.0 / 64.0)
                    # ============ conv this e (4 blocks) + silu ========
                    bl = slice(e * 4, e * 4 + 4)
                    cw = lambda tap: bc_t(
                        cvw[:, li * 32 + e * 4 + tap * 8:
                            li * 32 + e * 4 + tap * 8 + 4])
                    acc = dBf[:, 0:2048].bitcast(BF)     # [128, 4096] bf16
                    tmp = dBf[:, 2048:4096].bitcast(BF)
                    TT(mtv(acc), xpv[:, bl, 3:1027], cw(3), OP.mult)
                    for tap in (2, 1, 0):
                        TT(mtv(tmp), xpv[:, bl, tap:tap + 1024], cw(tap),
                           OP.mult)
                        TT(acc, acc, tmp, OP.add)
                    if has_cvb:
                        TT(mtv(acc), mtv(acc),
                           bc_t(cvb[:, li * 8 + e * 4: li * 8 + e * 4 + 4]),
                           OP.add)
                    ACT(xinq[:, e * 4096:(e + 1) * 4096], acc, AF.Silu,
                        scale=1.0 / 8.0)

                    # ============ in_proj z-half (this e) ==============
                    for m in range(4):
                        g = e * 4 + m
                        psz = pp.tile([128, 1024], F32, tag=f"p{m % 3}")
                        if "noin" in mode:
                            nc.vector.memset(psz[:], 0.0)
                        else:
                            for f in range(2):
                                for kt in range(2):
                                    off = ((li * 16 + 8 + g) * 2 + kt) * 256
                                    MM(psz[:, f * 512:(f + 1) * 512],
                                       winT[:, off:off + 256].rearrange(
                                           "p (i m) -> p i m", i=2),
                                       xiv(kt, f),
                                       start=(kt == 0), stop=(kt == 1),
                                       perf_mode=PMDR)
                        ACT(sz[:, g * 1024:(g + 1) * 1024], psz[:], AF.Silu,
                            scale=1.0 / 64.0)

                    # ============ x_proj partial (kt 2e, 2e+1) =========
                    for f in range(2):
                        for kth in range(2):
                            kt = 2 * e + kth
                            off = (li * 4 + kt) * 128
                            MM(psx[0:64, f * 512:(f + 1) * 512],
                               wxp[:, off:off + 128].rearrange(
                                   "p (i m) -> p i m", i=2),
                               xinv(kt, f),
                               start=(kt == 0), stop=(kt == 3),
                               perf_mode=PMDR)

                # ================= dbc evac (local, no CC) ==========
                TSM(dbcdt[0:32, :], psx[0:32, :], 1.0 / 64.0)
                TSM(bcst[32:64, :], psx[32:64, :], 1.0 / 64.0)
                nc.sync.dma_start(bcb_d[:], bcst[32:64, :])
                nc.sync.dma_start(
                    dBxh[0:1, 0:16384],
                    bcb_d[0:16, :].rearrange("(p a) b -> p (a b)", p=1))
                TSM(BCmat[0:1, 0:16384], dBxh[0:1, 0:16384], 8.0)
                nc.sync.dma_start(
                    dBxh[0:1, 0:16384],
                    bcb_d[16:32, :].rearrange("(p a) b -> p (a b)", p=1))
                TSM(BCmat[0:1, 16384:32768], dBxh[0:1, 0:16384], 1.0)
                nc.gpsimd.partition_broadcast(BCmat[:], BCmat[0:1, :])

                # ================= dt_proj + scan per e ==============
                for e in range(2):
                    dpre = dBxh[:].bitcast(BF)[:, 0:4096]
                    etmp = dBxh[:].bitcast(BF)[:, 4096:8192]
                    for m in range(4):
                        g = e * 4 + m
                        psd = pp.tile([128, 1024], F32, tag=f"p{m % 3}")
                        if "noin" in mode:
                            nc.vector.memset(psd[:], 0.0)
                        else:
                            for f in range(2):
                                MM(psd[:, f * 512:(f + 1) * 512],
                                   wdt[0:33, li * 1024 + g * 128:
                                       li * 1024 + (g + 1) * 128],
                                   dbcdt[0:33, f * 512:(f + 1) * 512],
                                   start=True, stop=True)
                        TSM(dpre[:, m * 1024:(m + 1) * 1024], psd[:], 1.0)
                    # exp/ln on the 4-block half -> lu[e]
                    ACT(etmp, dpre, AF.Exp)
                    ACT(lu[:], etmp, AF.Ln, bias=1.0)

                    kv = kvi[:, li * 16: li * 16 + 16]
                    for c in range(4):
                        g = e * 4 + c
                        gs = slice(g * 1024, (g + 1) * 1024)
                        if "noscan" in mode:
                            pyc = pp.tile([128, 1024], F32, tag=f"p{c % 3}")
                            nc.vector.memset(pyc[:], 0.0)
                        else:
                            TT(nlv(dA)[:, :, 1:1024],
                               bc_t(kv, 1024)[:, :, 1:1024],
                               lu[:, c * 1024 + 1:(c + 1) * 1024]
                               .unsqueeze(1).broadcast_to([128, N, 1023]),
                               OP.mult)
                            ACT(nlv(dA)[:, :, 1:1024], nlv(dA)[:, :, 1:1024],
                                AF.Exp)
                            TT(utc[:], lu[:, c * 1024:(c + 1) * 1024],
                               xinq[:, gs], OP.mult)
                            TT(nlv(dBxh),
                               utc[:].unsqueeze(1).broadcast_to(
                                   [128, N, 1024]),
                               BCmat[:, 0:16384].rearrange(
                                   "p (n l) -> p n l", n=N),
                               OP.mult)
                            nc.vector.tensor_tensor_scan(
                                dBxh[:], dA[:], dBxh[:], 0.0,
                                OP.mult, OP.add)
                            TT(nlv(dBxh), nlv(dBxh),
                               BCmat[:, 16384:32768].rearrange(
                                   "p (n l) -> p n l", n=N),
                               OP.mult)
                            pyc = pp.tile([128, 1024], F32, tag=f"p{c % 3}")
                            nc.vector.tensor_reduce(
                                pyc[:],
                                dBxh[:].rearrange("p (n l) -> p l n", n=N),
                                mybir.AxisListType.X, OP.add)
                        # gate: y3 = (y + xin*D*32) * sz
                        TT(utc[:].unsqueeze(1), xinq[:, gs].unsqueeze(1),
                           bc_t(Dv[:, li * 8 + g: li * 8 + g + 1]), OP.mult)
                        TT(utc[:], utc[:], pyc[:], OP.add)
                        TT(y3q[:, gs], utc[:], sz[:, gs], OP.mult)

                # ================= out_proj (full contraction) ======
                for mo in range(4):
                    pso = pp.tile([128, 1024], F32, tag=f"p{mo % 3}")
                    if "noout" in mode:
                        nc.vector.memset(pso[:], 0.0)
                    else:
                        for kt in range(4):
                            for f in range(2):
                                off = ((li * 4 + mo) * 4 + kt) * 256
                                MM(pso[:, f * 512:(f + 1) * 512],
                                   wout[:, off:off + 256].rearrange(
                                       "p (i m) -> p i m", i=2),
                                   y3vv(kt, f),
                                   start=(kt == 0), stop=(kt == 3),
                                   perf_mode=PMDR)
                    TSM(bob[:, mo * 1024:(mo + 1) * 1024], pso[:],
                        1.0 / 256.0)
                nc.vector.scalar_tensor_tensor(
                    xW[:], bob[:], 1.0 / 64.0, xT[:], OP.mult, OP.add)

            # ================= head =================
            TT(mtv(dBf[:, 0:4096]), mtv(xW[:]), bc_t(fcp[:]), OP.mult)
            nc.vector.tensor_reduce(
                dBf[:, 4096:5120],
                dBf[:, 0:4096].rearrange("p (dc t) -> p t dc", dc=4),
                mybir.AxisListType.X, OP.add)
            nc.gpsimd.partition_all_reduce(
                dBf[:, 5120:6144], dBf[:, 4096:5120], 128, RED.add)
            ACT(dBf[0:1, 6144:7168], dBf[0:1, 5120:6144],
                AF.Sigmoid, bias=fcb[:])
            nc.sync.dma_start(out_d[:], dBf[0:1, 6144:7168])
        rep_ctx.__exit__(None, None, None)

    nc.finalize()
    return nc


def _pack_core(inp, b):
    m = {}
    xt = np.asarray(inp["x"])[b].T.astype(np.float32)  # [512, 1024]
    m["xT"] = np.ascontiguousarray(
        xt.reshape(4, 128, 1024).transpose(1, 0, 2).reshape(128, 4096)
    ).astype(BF16)

    winT = np.zeros((128, 16384), F8)
    for li in range(NL):
        Wc = (np.asarray(inp["in_proj_w"])[li].astype(np.float32)
              * np.asarray(inp["norm_w"])[li][None, :].astype(np.float32))
        arr = (Wc * 64.0).reshape(16, 128, 4, 128).transpose(3, 0, 2, 1
                                                             ).reshape(128, 8192)
        winT[:, li * 8192:(li + 1) * 8192] = arr.astype(F8)
    m["winT"] = winT

    wout = np.zeros((128, 8192), F8)
    for li in range(NL):
        Wol = np.asarray(inp["out_proj_w"])[li].astype(np.float32)  # [512,1024]
        arr = (Wol * 64.0).reshape(4, 128, 8, 128).transpose(3, 0, 2, 1
                                                             ).reshape(128, 4096)
        wout[:, li * 4096:(li + 1) * 4096] = arr.astype(F8)
    m["wout"] = wout

    wxp = np.zeros((128, 1024), F8)
    for li in range(NL):
        Wxl = np.asarray(inp["x_proj_w"])[li].astype(np.float32).copy()
        Wxl[48:64] *= 32.0
        arr = (Wxl * 64.0).T.reshape(8, 128, 64).transpose(1, 0, 2
                                                           ).reshape(128, 512)
        wxp[:, li * 512:(li + 1) * 512] = arr.astype(F8)
    m["wxp"] = wxp

    wdt = np.zeros((33, 2048), BF16)
    for li in range(NL):
        Wdl = np.asarray(inp["dt_w"])[li].astype(np.float32)  # [1024, 32]
        wdt[0:32, li * 1024:(li + 1) * 1024] = Wdl.T.astype(BF16)
        wdt[32, li * 1024:(li + 1) * 1024] = \
            np.asarray(inp["dt_b"])[li].astype(np.float32).astype(BF16)
    m["wdt"] = wdt

    cvw = np.zeros((128, 64), np.float32)
    for li in range(NL):
        cwl = np.asarray(inp["conv_w"])[li][:, 0, :].astype(np.float32)
        # [1024 ch, 4 taps] -> per tap: [8 m, 128] -> cols tap*8 + m
        cvw[:, li * 32:(li + 1) * 32] = \
            cwl.reshape(8, 128, 4).transpose(1, 2, 0).reshape(128, 32)
    m["cvw"] = cvw

    def cols16(v, scale=1.0):
        out = np.zeros((128, 16), np.float32)
        for li in range(NL):
            out[:, li * 8:(li + 1) * 8] = np.asarray(v)[li].astype(
                np.float32).reshape(8, 128).T * scale
        return out

    m["cvb"] = cols16(inp["conv_b"], 8.0)
    m["Dv"] = cols16(inp["D"], 256.0)

    kvi = np.zeros((128, 32), np.float32)
    for li in range(NL):
        A = -np.exp(np.asarray(inp["A_log"])[li].astype(np.float64))
        kvi[:, li * 16:(li + 1) * 16] = A[0].astype(np.float32)[None, :]
    m["kvi"] = kvi

    fcp = np.zeros((128, 4), np.float32)
    fw = np.asarray(inp["fc_w"]).reshape(-1).astype(np.float32)
    for dc in range(4):
        fcp[:, dc] = fw[dc * 128:(dc + 1) * 128]
    m["fcp"] = fcp
    m["fcb"] = np.array([[float(np.asarray(inp["fc_b"]).reshape(-1)[0])]],
                        np.float32)
    return m


def kernel(**inputs):
    global LAST_RUN_S
    kvals = []
    for li in range(NL):
        A = -np.exp(np.asarray(inputs["A_log"])[li].astype(np.float64))
        a0 = A[0]
        assert np.abs(A - a0[None, :]).max() <= 1e-6 * np.abs(a0).max(), \
            "A not uniform across channels"
        kvals.append(tuple(float(v) for v in a0))
    has_cvb = bool(np.abs(np.asarray(inputs["conv_b"])).max() > 0)
    key = (REPEAT, ABLATE, has_cvb, tuple(kvals))
    if key not in _CACHE:
        _CACHE[key] = _build(REPEAT, kvals, has_cvb, ABLATE)
    nc = _CACHE[key]
    in_maps = [_pack_core(inputs, core // 2) for core in range(8)]
    t0 = time.time()
    res = run_bass_kernel_spmd(nc, in_maps, list(range(8)))
    LAST_RUN_S = time.time() - t0
    out = np.concatenate([
        np.asarray(res.results[2 * b]["out"], np.float32).reshape(-1)
        for b in range(B)])
    return out
